# revision 9
# baseline (speedup 1.0000x reference)
"""Per-channel Linear(seq->pred) over channels, 8-core channel-parallel Trainium2 kernel.

Math: y[b,p,c] = sum_s x[b,s,c] * W[c,p,s] + bias[c,p]

Strategy:
  - Shard channels C=321 across 8 cores (pad to 328 = 8*41).
  - Host-side re-layout (contraction padded to 726 = 6*121 rows):
      wt[c,s,p] = W[c,p,s] for s<720, wt[c,720,p] = bias[c,p], rows 721+ zero
      xt[c,s,b] = x[b,s,c] for s<720, xt[c,720,b] = 1.0,        rows 721+ zero
    so bias is folded into the contraction and the K dim splits into 6
    uniform chunks of 121 (one 3-dim DMA AP covers a whole channel pair).
  - Per channel: Y_c[b,p] = sum_k xT_chunk[k].T @ wT_chunk[k], accumulated in
    PSUM over the 6 K-chunks. lhsT = xT chunk [121,64] (stationary),
    rhs = wT chunk [121,720] streamed as N = 512 + 208 (PSUM bank limit).
  - Two channels share one PSUM tile via PE column tiling: channel A in
    output partitions 0:64, channel B in 64:128, matmuls interleaved so the
    two 64-wide column groups stream concurrently.
  - Result copied PSUM->SBUF (DVE + ACT split) and DMA'd out as y[c,b,p].
"""

import numpy as np

import concourse.bacc as bacc
import concourse.mybir as mybir
import concourse.tile as tile
from concourse.bass_utils import run_bass_kernel_spmd

F32 = mybir.dt.float32

B = 64          # batch
S = 720         # seq_len (contraction)
P = 720         # pred_len
C = 321         # channels
N_CORES = 8
CL = 41         # channels per core; 8*41 = 328 >= 321
CPAD = N_CORES * CL
KCH = 104       # K-chunk rows
NKCH = 7        # chunks per channel
SPAD = KCH * NKCH  # 726 padded contraction rows (720 data + bias + 5 zero)
NSPLIT = 512    # first matmul N (PSUM bank holds 512 f32)

_CACHE: dict = {}


def _build_module():
    nc = bacc.Bacc("TRN2", target_bir_lowering=False, debug=False,
                   num_devices=N_CORES)
    wt = nc.dram_tensor("wt", [CL, SPAD, P], F32, kind="ExternalInput").ap()
    xt = nc.dram_tensor("xt", [CL, SPAD, B], F32, kind="ExternalInput").ap()
    y = nc.dram_tensor("y", [CL, B, P], F32, kind="ExternalOutput").ap()

    with tile.TileContext(nc) as tc:
        with (
            tc.tile_pool(name="wp", bufs=3) as wp,
            tc.tile_pool(name="xp", bufs=3) as xp,
            tc.tile_pool(name="pp", bufs=3, space="PSUM") as pp,
            tc.tile_pool(name="op", bufs=3) as op,
        ):
            # process channels in pairs: two channels share one PSUM tile
            # (output partitions 0:64 and 64:128 -> PE column tiling).
            for c0 in range(0, CL, 2):
                pair = min(2, CL - c0)
                nch = pair * NKCH
                wbig = wp.tile([KCH, nch, P], F32, name=f"wbig{c0}", tag="wbig")
                xbig = xp.tile([KCH, nch, B], F32, name=f"xbig{c0}", tag="xbig")
                # (c, k) merge into one AP dim: c-step = SPAD*P = NKCH*(KCH*P)
                nc.sync.dma_start(
                    wbig[:],
                    wt[c0:c0 + pair].rearrange("c (k s) p -> s (c k) p", s=KCH))
                nc.sync.dma_start(
                    xbig[:],
                    xt[c0:c0 + pair].rearrange("c (k s) b -> s (c k) b", s=KCH))
                ps = pp.tile([pair * B, P], F32, name=f"ps{c0}", tag="ps")
                for k in range(NKCH):
                    st, sp = (k == 0), (k == NKCH - 1)
                    for half in range(pair):
                        ck = half * NKCH + k
                        lhsT = xbig[:, ck, :]
                        prow = half * B
                        nc.tensor.matmul(ps[prow:prow + B, 0:NSPLIT],
                                         lhsT, wbig[:, ck, 0:NSPLIT],
                                         start=st, stop=sp)
                        nc.tensor.matmul(ps[prow:prow + B, NSPLIT:P],
                                         lhsT, wbig[:, ck, NSPLIT:P],
                                         start=st, stop=sp)
                out = op.tile([pair * B, P], F32, name=f"out{c0}", tag="out")
                nc.vector.tensor_copy(out[:, 0:NSPLIT], ps[:, 0:NSPLIT])
                nc.scalar.copy(out[:, NSPLIT:P], ps[:, NSPLIT:P])
                nc.sync.dma_start(
                    y[c0:c0 + pair].rearrange("c b p -> (c b) p"), out[:])

    nc.compile()
    return nc


def _get_module():
    if "nc" not in _CACHE:
        _CACHE["nc"] = _build_module()
    return _CACHE["nc"]


def _prep_inputs(x, W, b):
    wt = np.zeros((CPAD, SPAD, P), dtype=np.float32)
    wt[:C, :S, :] = W.transpose(0, 2, 1)
    wt[:C, S, :] = b
    xt = np.zeros((CPAD, SPAD, B), dtype=np.float32)
    xt[:C, :S, :] = x.transpose(2, 1, 0)
    xt[:C, S, :] = 1.0
    in_maps = []
    for i in range(N_CORES):
        sl = slice(i * CL, (i + 1) * CL)
        in_maps.append({
            "wt": np.ascontiguousarray(wt[sl]),
            "xt": np.ascontiguousarray(xt[sl]),
        })
    return in_maps


def _gather(results):
    ys = np.concatenate([results[i]["y"] for i in range(N_CORES)], axis=0)
    return np.ascontiguousarray(ys[:C].transpose(1, 2, 0))


def run(x, W, b, **run_kwargs):
    """Full pipeline, returns (output, BassKernelResults)."""
    nc = _get_module()
    in_maps = _prep_inputs(np.asarray(x), np.asarray(W), np.asarray(b))
    res = run_bass_kernel_spmd(nc, in_maps, list(range(N_CORES)), **run_kwargs)
    return _gather(res.results), res


def kernel(x, W, b):
    out, _ = run(x, W, b)
    return out


# revision 10
# speedup vs baseline: 1.5558x; 1.5558x over previous
"""Per-channel Linear(seq->pred) over channels, 8-core channel-parallel Trainium2 kernel.

Math: y[b,p,c] = sum_s x[b,s,c] * W[c,p,s] + bias[c,p]

Strategy:
  - Shard channels C=321 across 8 cores (pad to 328 = 8*41).
  - Host-side re-layout into a flat row stream, 736 rows per channel
    (720 W^T rows + 1 bias row + 15 zero rows; 736 = 23*32 keeps every
    channel start 32-aligned inside 128-row strips):
      wtf[736*j + s, p] = W[c,p,s], wtf[736*j + 720, p] = bias[c,p]
      xtf[736*j + s, b] = x[b,s,c], xtf[736*j + 720, b] = 1.0
    so bias is folded into the contraction.
  - DMA moves uniform 128-row strips (always 128 partitions - required for
    full DMA bandwidth): one 8.5 MB DMA per quad of 4 channels (23 strips).
  - Per channel: Y_c[b,p] = sum_k xT_chunk[k].T @ wT_chunk[k], accumulated
    in PSUM. K-chunks follow the strip layout: a channel's 736 rows split
    into 32/64/96/128-row chunks at legal PE row offsets (0/32/64/96).
    lhsT = xT chunk [kk,64] (stationary), rhs = wT chunk [kk,720] streamed
    as N = 512 + 208 (PSUM bank limit).
  - Two channels share one PSUM tile via PE column tiling: channel A in
    output partitions 0:64, channel B in 64:128, matmuls interleaved so the
    two 64-wide column groups stream concurrently.
  - Result copied PSUM->SBUF (DVE + ACT split) and DMA'd out as y[c,b,p].
"""

import numpy as np

import concourse.bacc as bacc
import concourse.mybir as mybir
import concourse.tile as tile
from concourse.bass_utils import run_bass_kernel_spmd

F32 = mybir.dt.float32

B = 64          # batch
S = 720         # seq_len (contraction)
P = 720         # pred_len
C = 321         # channels
N_CORES = 8
CL = 41         # channels per core; 8*41 = 328 >= 321
CPAD = N_CORES * CL
SPC = 736       # padded rows per channel (720 data + bias + 15 zero), 23*32
NQ = CL // 4    # full quads of 4 channels (10)
QSTRIPS = SPC * 4 // 128          # 23 strips per quad
NSTRIP = (CL * SPC + 127) // 128  # 236 strips total (incl. final pad)
ROWS_PAD = NSTRIP * 128           # 30208 flat rows per core
NSPLIT = 512    # first matmul N (PSUM bank holds 512 f32)

_CACHE: dict = {}


def _chunks(j):
    """K-chunks for channel slot j inside its quad block: (strip, off, kk).

    Offsets within 128-row strips must be legal PE row-tile positions:
    kk<=32 at off in {0,32,64,96}; kk<=64 at {0,64}; kk<=128 at 0.
    """
    r, r1, out = SPC * j, SPC * (j + 1), []
    while r < r1:
        strip, off = r // 128, r % 128
        rem = min(128 - off, r1 - r)
        if off == 0:
            kk = min(128, rem)
        elif off == 64:
            kk = min(64, rem)
        else:  # 32 or 96
            kk = min(32, rem)
        out.append((strip, off, kk))
        r += kk
    return out


def _build_module():
    nc = bacc.Bacc("TRN2", target_bir_lowering=False, debug=False,
                   num_devices=N_CORES)
    wt = nc.dram_tensor("wt", [ROWS_PAD, P], F32, kind="ExternalInput").ap()
    xt = nc.dram_tensor("xt", [ROWS_PAD, B], F32, kind="ExternalInput").ap()
    y = nc.dram_tensor("y", [CL, B, P], F32, kind="ExternalOutput").ap()

    with tile.TileContext(nc) as tc:
        with (
            tc.tile_pool(name="wp", bufs=2) as wp,
            tc.tile_pool(name="xp", bufs=2) as xp,
            tc.tile_pool(name="pp", bufs=4, space="PSUM") as pp,
            tc.tile_pool(name="op", bufs=3) as op,
        ):
            def issue_pair(wq, xq, chans, slots):
                """Two channels -> one PSUM tile (col groups 0:64 / 64:128)."""
                npair = len(chans)
                ps = pp.tile([npair * B, P], F32, name=f"ps{chans[0]}", tag="ps")
                chunk_lists = [_chunks(slot) for slot in slots]
                for i in range(max(len(cl) for cl in chunk_lists)):
                    for half in range(npair):
                        cl = chunk_lists[half]
                        if i >= len(cl):
                            continue
                        si, o, kk = cl[i]
                        st, sp = (i == 0), (i == len(cl) - 1)
                        lhsT = xq[o:o + kk, si, :]
                        prow = half * B
                        nc.tensor.matmul(ps[prow:prow + B, 0:NSPLIT], lhsT,
                                         wq[o:o + kk, si, 0:NSPLIT],
                                         start=st, stop=sp,
                                         tile_position=(o, prow))
                        nc.tensor.matmul(ps[prow:prow + B, NSPLIT:P], lhsT,
                                         wq[o:o + kk, si, NSPLIT:P],
                                         start=st, stop=sp,
                                         tile_position=(o, prow))
                out = op.tile([npair * B, P], F32, name=f"out{chans[0]}", tag="out")
                nc.vector.tensor_copy(out[:, 0:NSPLIT], ps[:, 0:NSPLIT])
                nc.scalar.copy(out[:, NSPLIT:P], ps[:, NSPLIT:P])
                nc.sync.dma_start(
                    y[chans[0]:chans[0] + npair].rearrange("c b p -> (c b) p"),
                    out[:])

            def load_block(c0, nstrips):
                base = c0 * SPC  # row index, strip-aligned by construction
                wq = wp.tile([128, nstrips, P], F32, name=f"wq{c0}", tag="wq")
                xq = xp.tile([128, nstrips, B], F32, name=f"xq{c0}", tag="xq")
                nc.sync.dma_start(
                    wq[:], wt[base:base + nstrips * 128, :]
                    .rearrange("(k s) p -> s k p", s=128))
                nc.sync.dma_start(
                    xq[:], xt[base:base + nstrips * 128, :]
                    .rearrange("(k s) b -> s k b", s=128))
                return wq, xq

            for q in range(NQ):
                wq, xq = load_block(4 * q, QSTRIPS)
                issue_pair(wq, xq, [4 * q, 4 * q + 1], [0, 1])
                issue_pair(wq, xq, [4 * q + 2, 4 * q + 3], [2, 3])
            # leftover channel(s): CL - 4*NQ (= 1 for CL=41); block start is
            # strip-aligned since 4*SPC is a multiple of 128
            nleft = CL - 4 * NQ
            if nleft:
                lstrips = NSTRIP - NQ * QSTRIPS
                wq, xq = load_block(4 * NQ, lstrips)
                for c0 in range(0, nleft, 2):
                    npair = min(2, nleft - c0)
                    issue_pair(wq, xq, [4 * NQ + c0 + j for j in range(npair)],
                               [c0 + j for j in range(npair)])

    nc.compile()
    return nc


def _get_module():
    if "nc" not in _CACHE:
        _CACHE["nc"] = _build_module()
    return _CACHE["nc"]


def _prep_inputs(x, W, b):
    wt = np.zeros((CPAD, SPC, P), dtype=np.float32)
    wt[:C, :S, :] = W.transpose(0, 2, 1)
    wt[:C, S, :] = b
    xt = np.zeros((CPAD, SPC, B), dtype=np.float32)
    xt[:C, :S, :] = x.transpose(2, 1, 0)
    xt[:C, S, :] = 1.0
    in_maps = []
    pad = np.zeros((ROWS_PAD - CL * SPC, P), dtype=np.float32)
    padx = np.zeros((ROWS_PAD - CL * SPC, B), dtype=np.float32)
    for i in range(N_CORES):
        sl = slice(i * CL, (i + 1) * CL)
        in_maps.append({
            "wt": np.concatenate([wt[sl].reshape(CL * SPC, P), pad], axis=0),
            "xt": np.concatenate([xt[sl].reshape(CL * SPC, B), padx], axis=0),
        })
    return in_maps


def _gather(results):
    ys = np.concatenate([results[i]["y"] for i in range(N_CORES)], axis=0)
    return np.ascontiguousarray(ys[:C].transpose(1, 2, 0))


def run(x, W, b, **run_kwargs):
    """Full pipeline, returns (output, BassKernelResults)."""
    nc = _get_module()
    in_maps = _prep_inputs(np.asarray(x), np.asarray(W), np.asarray(b))
    res = run_bass_kernel_spmd(nc, in_maps, list(range(N_CORES)), **run_kwargs)
    return _gather(res.results), res


def kernel(x, W, b):
    out, _ = run(x, W, b)
    return out


# revision 11
# speedup vs baseline: 1.8521x; 1.1905x over previous
"""Per-channel Linear(seq->pred) over channels, 8-core channel-parallel Trainium2 kernel.

Math: y[b,p,c] = sum_s x[b,s,c] * W[c,p,s] + bias[c,p]

Strategy:
  - Shard channels C=321 across 8 cores (pad to 328 = 8*41).
  - Host-side re-layout (contraction padded to 726 = 6*121 rows):
      wt[c,s,p] = W[c,p,s] for s<720, wt[c,720,p] = bias[c,p], rows 721+ zero
      xt[c,s,b] = x[b,s,c] for s<720, xt[c,720,b] = 1.0,        rows 721+ zero
    so bias is folded into the contraction and the K dim splits into 6
    uniform chunks of 121 (one 3-dim DMA AP covers a whole channel pair).
  - Per channel: Y_c[b,p] = sum_k xT_chunk[k].T @ wT_chunk[k], accumulated in
    PSUM over the 6 K-chunks. lhsT = xT chunk [121,64] (stationary),
    rhs = wT chunk [121,720] streamed as N = 512 + 208 (PSUM bank limit).
  - Two channels share one PSUM tile via PE column tiling: channel A in
    output partitions 0:64, channel B in 64:128, matmuls interleaved so the
    two 64-wide column groups stream concurrently.
  - Result copied PSUM->SBUF (DVE + ACT split) and DMA'd out as y[c,b,p].
"""

import numpy as np

import concourse.bacc as bacc
import concourse.mybir as mybir
import concourse.tile as tile
from concourse.bass_utils import run_bass_kernel_spmd

F32 = mybir.dt.float32

B = 64          # batch
S = 720         # seq_len (contraction)
P = 720         # pred_len
C = 321         # channels
N_CORES = 8
CL = 41         # channels per core; 8*41 = 328 >= 321
CPAD = N_CORES * CL
KCH = 128       # K-chunk rows
NKCH = 6        # chunks per channel
SPAD = KCH * NKCH  # 726 padded contraction rows (720 data + bias + 5 zero)
NSPLIT = 512    # first matmul N (PSUM bank holds 512 f32)

_CACHE: dict = {}


def _build_module():
    nc = bacc.Bacc("TRN2", target_bir_lowering=False, debug=False,
                   num_devices=N_CORES)
    wt = nc.dram_tensor("wt", [CL, SPAD, P], F32, kind="ExternalInput").ap()
    xt = nc.dram_tensor("xt", [CL, SPAD, B], F32, kind="ExternalInput").ap()
    y = nc.dram_tensor("y", [CL, B, P], F32, kind="ExternalOutput").ap()

    with tile.TileContext(nc) as tc:
        with (
            tc.tile_pool(name="wp", bufs=3) as wp,
            tc.tile_pool(name="xp", bufs=3) as xp,
            tc.tile_pool(name="pp", bufs=3, space="PSUM") as pp,
            tc.tile_pool(name="op", bufs=3) as op,
        ):
            # process channels in pairs: two channels share one PSUM tile
            # (output partitions 0:64 and 64:128 -> PE column tiling).
            for c0 in range(0, CL, 2):
                pair = min(2, CL - c0)
                nch = pair * NKCH
                wbig = wp.tile([KCH, nch, P], F32, name=f"wbig{c0}", tag="wbig")
                xbig = xp.tile([KCH, nch, B], F32, name=f"xbig{c0}", tag="xbig")
                # (c, k) merge into one AP dim: c-step = SPAD*P = NKCH*(KCH*P)
                nc.sync.dma_start(
                    wbig[:],
                    wt[c0:c0 + pair].rearrange("c (k s) p -> s (c k) p", s=KCH))
                nc.sync.dma_start(
                    xbig[:],
                    xt[c0:c0 + pair].rearrange("c (k s) b -> s (c k) b", s=KCH))
                ps = pp.tile([pair * B, P], F32, name=f"ps{c0}", tag="ps")
                for k in range(NKCH):
                    st, sp = (k == 0), (k == NKCH - 1)
                    for half in range(pair):
                        ck = half * NKCH + k
                        lhsT = xbig[:, ck, :]
                        prow = half * B
                        nc.tensor.matmul(ps[prow:prow + B, 0:NSPLIT],
                                         lhsT, wbig[:, ck, 0:NSPLIT],
                                         start=st, stop=sp)
                        nc.tensor.matmul(ps[prow:prow + B, NSPLIT:P],
                                         lhsT, wbig[:, ck, NSPLIT:P],
                                         start=st, stop=sp)
                out = op.tile([pair * B, P], F32, name=f"out{c0}", tag="out")
                nc.vector.tensor_copy(out[:, 0:NSPLIT], ps[:, 0:NSPLIT])
                nc.scalar.copy(out[:, NSPLIT:P], ps[:, NSPLIT:P])
                nc.sync.dma_start(
                    y[c0:c0 + pair].rearrange("c b p -> (c b) p"), out[:])

    nc.compile()
    return nc


def _get_module():
    if "nc" not in _CACHE:
        _CACHE["nc"] = _build_module()
    return _CACHE["nc"]


def _prep_inputs(x, W, b):
    wt = np.zeros((CPAD, SPAD, P), dtype=np.float32)
    wt[:C, :S, :] = W.transpose(0, 2, 1)
    wt[:C, S, :] = b
    xt = np.zeros((CPAD, SPAD, B), dtype=np.float32)
    xt[:C, :S, :] = x.transpose(2, 1, 0)
    xt[:C, S, :] = 1.0
    in_maps = []
    for i in range(N_CORES):
        sl = slice(i * CL, (i + 1) * CL)
        in_maps.append({
            "wt": np.ascontiguousarray(wt[sl]),
            "xt": np.ascontiguousarray(xt[sl]),
        })
    return in_maps


def _gather(results):
    ys = np.concatenate([results[i]["y"] for i in range(N_CORES)], axis=0)
    return np.ascontiguousarray(ys[:C].transpose(1, 2, 0))


def run(x, W, b, **run_kwargs):
    """Full pipeline, returns (output, BassKernelResults)."""
    nc = _get_module()
    in_maps = _prep_inputs(np.asarray(x), np.asarray(W), np.asarray(b))
    res = run_bass_kernel_spmd(nc, in_maps, list(range(N_CORES)), **run_kwargs)
    return _gather(res.results), res


def kernel(x, W, b):
    out, _ = run(x, W, b)
    return out
